# revision 20
# baseline (speedup 1.0000x reference)
"""Trainium2 Bass kernel for stacked-head state attention (nn_ARCStacked).

Problem (hardcoded shapes):
  x: (2, 2304, 2048) f32; six weights (16, 2048, 128) f32; scaling (16,) f32.
  T = 2304 = 128 state + 2048 core + 128 state tokens. Per (batch b, head h):
    q/k/v = l2norm(x @ w[h]) per token (state weights for the two 128-token
    state segments, core weights for the middle 2048), causal attention with
    the extra rule that the last 128 rows can't attend to the first 128 cols,
    out = softmax(scale_h * q @ k^T + mask) @ v  -> (2, 16, 2304, 128) f32.

Sharding: 8 cores = 2 batches x 4 head-groups (4 heads each); outputs are
disjoint -> no collectives.

Per-core structure (v2): loop over five 512-token chunks J (last is 256).
  A_qk(J): q^T/k^T projected directly in [dk, token] layout (stationary = the
    head's weight chunk, moving = x^T), L2 norms via an all-ones [128,128]
    stationary matmul on the squared values (broadcast column sums), so the
    per-token normalizer never needs a partition reduction on DVE.
  A_v(J): v in token-major layout (norm on the free axis via ACT square+accum),
    stored with a ones column appended -> attn @ [v|1] yields the softmax
    denominator inside the same PSUM accumulation.
  B(J): attention for q-tiles of J (causal: only needs k/v tiles <= J). Scores
    are computed transposed (s^T[kt, qt]) so the exp blocks feed attn@v as the
    stationary operand directly. |s| <= scale (unit vectors) -> no max
    subtraction. scale_h applied via the Exp activation's scale argument.
    4 consecutive kt-blocks pack one PSUM bank -> one [128,<=512] Exp per group.
"""

import numpy as np
import ml_dtypes

B = 2
H = 16
T = 2304
D = 2048
DK = 128
NT = T // 128   # 18 token tiles; tiles 0 and 17 are the state segments
NCH = D // 128  # 16 contraction chunks
HG = 4          # heads per core
N_CORES = 8
NJ = 5          # 512-token chunks (last is 256)

_NC = None


def build_nc():
    global _NC
    if _NC is not None:
        return _NC

    from contextlib import ExitStack

    import concourse.tile as tile
    from concourse import bacc, mybir
    from concourse.masks import make_upper_triangular

    bf16 = mybir.dt.bfloat16
    f32 = mybir.dt.float32
    fp8 = mybir.dt.float8e4
    DR = mybir.MatmulPerfMode.DoubleRow
    AF = mybir.ActivationFunctionType

    import concourse.bacc as _bacc_mod
    import concourse.hw_specs as _hw_specs

    if not getattr(_hw_specs, "_act_tables_patched", False):
        _orig_gat = _hw_specs.get_activation_tables

        def _gat_one_set(arch):
            t = _orig_gat(arch)
            out, seen = {}, False
            for nm, funcs in t.items():
                if nm == "natural_log_exp_and_others":
                    seen = True
                    out[nm] = funcs
                else:
                    out[nm] = funcs if seen else set()
            return out

        _hw_specs.get_activation_tables = _gat_one_set
        _bacc_mod.get_activation_tables = _gat_one_set
        _hw_specs._act_tables_patched = True

    nc = bacc.Bacc()
    xt = nc.declare_dram_parameter("xt", [128, NCH // 2, 2, T], fp8, isOutput=False)
    w_params = {
        name: nc.declare_dram_parameter(name, [128, NCH // 2, 2, HG * 128], fp8,
                                        isOutput=False)
        for name in ("wq", "wk", "wqs", "wks")
    }
    w_params.update({
        name: nc.declare_dram_parameter(name, [128, NCH, HG * 128], bf16,
                                        isOutput=False)
        for name in ("wv", "wvs")
    })
    xtv = nc.declare_dram_parameter("xtv", [128, NT, NCH, 128], bf16,
                                    isOutput=False)
    scal = nc.declare_dram_parameter("scal", [128, HG], f32, isOutput=False)
    out = nc.declare_dram_parameter("out", [HG, T, DK], f32, isOutput=True)

    with tile.TileContext(nc) as tc, ExitStack() as ctx:
        consts = ctx.enter_context(tc.tile_pool(name="consts", bufs=1))
        # triT[i, j] = 1 iff i <= j: allowed entries of a transposed-scores
        # diagonal block (kt_local <= qt_local)
        triT = consts.tile([128, 128], bf16)
        make_upper_triangular(nc, triT[:], val=1.0, diag=True)
        ones128 = consts.tile([128, 128], bf16)
        nc.vector.memset(ones128[:], 1.0)
        scal_s = consts.tile([128, HG], f32)
        nc.sync.dma_start(scal_s[:], scal[:])

        xpool0 = ctx.enter_context(tc.tile_pool(name="x", bufs=3))
        xs0 = xpool0.tile([128, NCH // 2, 2, 512], fp8, tag="xs")
        nc.sync.dma_start(xs0[:], xt[:, :, :, 0:512])

        wpool = ctx.enter_context(tc.tile_pool(name="w", bufs=1))
        w_s = {}
        for name in ("wq", "wk", "wqs", "wks"):
            w_s[name] = wpool.tile([128, NCH // 2, 2, HG * 128], fp8, tag=name,
                                   name=name)
        for name in ("wv", "wvs"):
            w_s[name] = wpool.tile([128, NCH, HG * 128], bf16, tag=name,
                                   name=name)
        for pair in (("wqs", "wq"), ("wks", "wk")):
            for c2 in range(0, NCH // 2, 2):
                for name in pair:
                    nc.sync.dma_start(w_s[name][:, c2:c2 + 2],
                                      w_params[name][:, c2:c2 + 2])
        for name in ("wvs", "wv"):
            for c4 in range(0, NCH, 4):
                nc.sync.dma_start(w_s[name][:, c4:c4 + 4],
                                  w_params[name][:, c4:c4 + 4])

        big = ctx.enter_context(tc.tile_pool(name="big", bufs=1))
        kT_s = big.tile([128, HG, T], bf16, tag="kT")          # [dk, h, t]
        v_s = big.tile([128, HG, NT, 129], bf16, tag="v")      # [tl, h, ti, dv|1]
        nc.vector.memset(v_s[:, :, :, 128:129], 1.0)
        # per-(token%128, head, k-tile) exp scale: scale_h / ||k_token||
        snk_s = big.tile([128, HG, NT], f32, tag="snk")

        xpool = xpool0
        xvpool = ctx.enter_context(tc.tile_pool(name="xv", bufs=2))
        qpool = ctx.enter_context(tc.tile_pool(name="q", bufs=2))

        with (
            tc.tile_pool(name="psA", bufs=2, space="PSUM") as psA,
            tc.tile_pool(name="misc", bufs=2, space="PSUM") as misc,
            tc.tile_pool(name="psO", bufs=4, space="PSUM") as psO,
            tc.tile_pool(name="wk", bufs=3) as wk,
            tc.tile_pool(name="exw", bufs=6) as exw,
            tc.tile_pool(name="outp", bufs=3) as outp,
        ):
            for J in range(NJ):
                ntj = 4 if J < 4 else 2
                W = ntj * 128
                # x chunk: [128=d%128, tile, c, tl]
                if J == 0:
                    xs = xs0
                else:
                    xs = xpool.tile([128, NCH // 2, 2, 512], fp8, tag="xs")
                    nc.sync.dma_start(xs[:, :, :, :W],
                                      xt[:, :, :, 512 * J:512 * J + W])
                xv = xvpool.tile([128, 4, NCH, 128], bf16, tag="xv")
                nc.sync.dma_start(xv[:, :ntj], xtv[:, 4 * J:4 * J + ntj])
                qT_j = qpool.tile([128, HG, 512], bf16, tag="qT")

                state_w = {0: True, NJ - 1: True}.get(J, False)
                # tiles 0 and 17 are state; J=0 has tiles 0..3 (mixed), J=4 has
                # 16,17 (mixed) -> choose weights per token tile for v; for q/k
                # the 512-wide matmul spans tiles with different weights, so
                # split the contraction per token tile instead: do matmuls per
                # (c, tile-group of same kind).

                # ---- A_q: transposed projection + matmul-based norm ----
                for kind, wc, ws, dest in (("q", "wq", "wqs", None),):
                    for h in range(HG):
                        ps = psA.tile([128, 512], f32, tag="proj")
                        for c in range(NCH // 2):
                            for lo, hi, wname in _wgroups(J, ntj, wc, ws):
                                nc.tensor.matmul(
                                    ps[:, lo * 128:hi * 128],
                                    w_s[wname][:, c, :, h * 128:(h + 1) * 128],
                                    xs[:, c, :, lo * 128:hi * 128],
                                    start=(c == 0), stop=(c == NCH // 2 - 1),
                                    perf_mode=DR,
                                )
                        pc = wk.tile([128, 512], bf16, tag="pc")
                        nc.vector.tensor_copy(pc[:, :W], ps[:, :W])
                        sq = wk.tile([128, 512], bf16, tag="sq")
                        nc.vector.tensor_mul(sq[:, :W], pc[:, :W], pc[:, :W])
                        n2 = misc.tile([128, 512], f32, tag="mb")
                        nc.tensor.matmul(n2[:, :W], ones128[:], sq[:, :W],
                                         start=True, stop=True)
                        sn = wk.tile([128, 512], f32, tag="sn")
                        nc.scalar.activation(sn[:, :W], n2[:, :W], AF.Ln)
                        nc.scalar.activation(sn[:, :W], sn[:, :W], AF.Exp,
                                             scale=-0.5)
                        nc.vector.tensor_mul(qT_j[:, h, :W], pc[:, :W],
                                             sn[:, :W])

                # ---- A_k: token-major k; raw k^T via DMA transpose; norm
                # factor folded into the attention Exp's per-partition scale --
                for tl in range(ntj):
                    i = 4 * J + tl
                    kname = "wks" if i in (0, NT - 1) else "wk"
                    pk = psA.tile([128, 512], f32, tag="proj")
                    for c in range(NCH // 2):
                        nc.tensor.matmul(
                            pk[:], xs[:, c, :, tl * 128:(tl + 1) * 128],
                            w_s[kname][:, c], start=(c == 0),
                            stop=(c == NCH // 2 - 1), perf_mode=DR,
                        )
                    kc = wk.tile([128, 512], bf16, tag="kc")
                    nc.vector.tensor_copy(kc[:], pk[:])
                    sqk = wk.tile([128, 512], f32, tag="sqv")
                    nc.vector.tensor_mul(sqk[:], kc[:], kc[:])
                    n2k = wk.tile([128, HG], f32, tag="n2k")
                    nc.vector.reduce_sum(
                        out=n2k[:],
                        in_=sqk[:].rearrange("p (h e) -> p h e", h=HG),
                        axis=mybir.AxisListType.X,
                    )
                    nc.scalar.activation(n2k[:], n2k[:], AF.Ln)
                    nc.scalar.activation(n2k[:], n2k[:], AF.Exp, scale=-0.5)
                    nc.vector.tensor_mul(snk_s[:, :, i], n2k[:], scal_s[:])
                    for h in range(HG):
                        nc.sync.dma_start_transpose(
                            kT_s[:, h, i * 128:(i + 1) * 128],
                            kc[:, h * 128:(h + 1) * 128],
                        )

                # ---- A_v: token-major v + ones column ----
                for tl in range(ntj):
                    i = 4 * J + tl
                    wname = "wvs" if i in (0, NT - 1) else "wv"
                    pv = psA.tile([128, 512], f32, tag="proj")
                    for c in range(NCH):
                        nc.tensor.matmul(
                            pv[:], xv[:, tl, c, :], w_s[wname][:, c],
                            start=(c == 0), stop=(c == NCH - 1),
                        )
                    vc = wk.tile([128, 512], bf16, tag="vc")
                    nc.vector.tensor_copy(vc[:], pv[:])
                    sqv = wk.tile([128, 512], f32, tag="sqv")
                    nc.vector.tensor_mul(sqv[:], vc[:], vc[:])
                    n2v = wk.tile([128, HG], f32, tag="n2v")
                    nc.vector.reduce_sum(
                        out=n2v[:],
                        in_=sqv[:].rearrange("p (h e) -> p h e", h=HG),
                        axis=mybir.AxisListType.X,
                    )
                    nc.scalar.activation(n2v[:], n2v[:], AF.Ln)
                    nc.scalar.activation(n2v[:], n2v[:], AF.Exp, scale=-0.5)
                    for h in range(HG):
                        nc.vector.tensor_scalar_mul(
                            v_s[:, h, i, 0:128],
                            vc[:, h * 128:(h + 1) * 128],
                            n2v[:, h:h + 1],
                        )

                # ---- B: attention for q-tiles of this chunk. Scores for one
                # k-tile cover all (allowed) q-tiles of the chunk at once;
                # k's norm (and scale_h) ride the Exp scale per partition. ----
                for h in range(HG):
                    po2s = [psO.tile([128, 129], f32, tag="po", name=f"po{J}_{h}_{p2}")
                            for p2 in range(ntj)]
                    started = [False] * ntj
                    for ki in range(4 * J + ntj):
                        lo = max(ki - 4 * J, 0)
                        width = W - lo * 128
                        pss = misc.tile([128, 512], f32, tag="mb")
                        nc.tensor.matmul(
                            pss[:, :width],
                            kT_s[:, h, ki * 128:(ki + 1) * 128],
                            qT_j[:, h, lo * 128:W],
                            start=True, stop=True,
                        )
                        ex = exw.tile([128, 512], bf16, tag="ex")
                        nc.scalar.activation(ex[:, :width], pss[:, :width],
                                             AF.Exp,
                                             scale=snk_s[:, h, ki:ki + 1])
                        if ki >= 4 * J:
                            nc.vector.tensor_mul(ex[:, 0:128], ex[:, 0:128],
                                                 triT[:])
                        for ql in range(lo, ntj):
                            qi = 4 * J + ql
                            if qi == NT - 1 and ki == 0:
                                continue
                            sl = ql - lo
                            tgt = po2s[ql][:]
                            nc.tensor.matmul(
                                tgt, ex[:, sl * 128:(sl + 1) * 128],
                                v_s[:, h, ki, :],
                                start=(not started[ql]), stop=(ki == qi),
                            )
                            started[ql] = True
                    for ql in range(ntj):
                        qi = 4 * J + ql
                        po = po2s[ql][:]
                        den = exw.tile([128, 1], f32, tag="den")
                        nc.vector.reciprocal(den[:], po[:, 128:129])
                        ot = outp.tile([128, 128], f32, tag="ot")
                        nc.vector.tensor_scalar_mul(ot[:], po[:, 0:128], den[:])
                        nc.sync.dma_start(out[h, qi * 128:(qi + 1) * 128, :],
                                          ot[:])

    nc.finalize()
    _NC = nc
    return nc


def _wgroups(J, ntj, wc, ws):
    """Token-tile ranges [lo, hi) within chunk J sharing one weight tensor."""
    if J == 0:
        return [(0, 1, ws), (1, ntj, wc)]
    if J == NJ - 1:
        return [(0, ntj - 1, wc), (ntj - 1, ntj, ws)]
    return [(0, ntj, wc)]


def _shard_inputs(x, w_q, w_k, w_v, w_q_state, w_k_state, w_v_state,
                  scaling_factor):
    bf16 = ml_dtypes.bfloat16

    fp8 = ml_dtypes.float8_e4m3

    def prep_x(xb):
        xt = np.ascontiguousarray(xb.T)                       # (D, T)
        xt = xt.reshape(NCH // 2, 2, 128, T).transpose(2, 0, 1, 3)
        return np.ascontiguousarray(xt.astype(fp8))

    def prep_xv(xb):
        xt = np.ascontiguousarray(xb.T)
        xt = xt.reshape(NCH, 128, NT, 128).transpose(1, 2, 0, 3)
        return np.ascontiguousarray(xt.astype(bf16))

    def prep_w(w, g):
        w4 = w[HG * g:HG * (g + 1)].transpose(1, 0, 2).reshape(D, HG * DK)
        w4 = w4.reshape(NCH // 2, 2, 128, HG * DK).transpose(2, 0, 1, 3)
        return np.ascontiguousarray(w4.astype(fp8))

    def prep_wv(w, g):
        w4 = w[HG * g:HG * (g + 1)].transpose(1, 0, 2).reshape(D, HG * DK)
        w4 = w4.reshape(NCH, 128, HG * DK).transpose(1, 0, 2)
        return np.ascontiguousarray(w4.astype(bf16))

    xbs = [np.asarray(x[b], dtype=np.float32) for b in range(B)]
    xts = [prep_x(xb) for xb in xbs]
    xtvs = [prep_xv(xb) for xb in xbs]
    in_maps = []
    for core in range(N_CORES):
        b, g = divmod(core, N_CORES // B)
        m = {"xt": xts[b], "xtv": xtvs[b]}
        for name, w in (("wq", w_q), ("wk", w_k),
                        ("wqs", w_q_state), ("wks", w_k_state)):
            m[name] = prep_w(np.asarray(w, dtype=np.float32), g)
        for name, w in (("wv", w_v), ("wvs", w_v_state)):
            m[name] = prep_wv(np.asarray(w, dtype=np.float32), g)
        sc = np.asarray(scaling_factor, dtype=np.float32)[HG * g:HG * (g + 1)]
        m["scal"] = np.ascontiguousarray(
            np.broadcast_to(sc[None, :], (128, HG)).astype(np.float32))
        in_maps.append(m)
    return in_maps


def run_on_cores(in_maps, **kwargs):
    from concourse.bass_utils import run_bass_kernel_spmd

    nc = build_nc()
    return run_bass_kernel_spmd(nc, in_maps, list(range(N_CORES)), **kwargs)


def kernel(x, w_q, w_k, w_v, w_q_state, w_k_state, w_v_state, scaling_factor):
    in_maps = _shard_inputs(x, w_q, w_k, w_v, w_q_state, w_k_state, w_v_state,
                            scaling_factor)
    res = run_on_cores(in_maps)
    full = np.empty((B, H, T, DK), dtype=np.float32)
    for core in range(N_CORES):
        b, g = divmod(core, N_CORES // B)
        full[b, HG * g:HG * (g + 1)] = res.results[core]["out"]
    return full


# revision 22
# speedup vs baseline: 1.2001x; 1.2001x over previous
"""Trainium2 Bass kernel for stacked-head state attention (nn_ARCStacked).

Problem (hardcoded shapes):
  x: (2, 2304, 2048) f32; six weights (16, 2048, 128) f32; scaling (16,) f32.
  T = 2304 = 128 state + 2048 core + 128 state tokens. Per (batch b, head h):
    q/k/v = l2norm(x @ w[h]) per token (state weights for the two 128-token
    state segments, core weights for the middle 2048), causal attention with
    the extra rule that the last 128 rows can't attend to the first 128 cols,
    out = softmax(scale_h * q @ k^T + mask) @ v  -> (2, 16, 2304, 128) f32.

Sharding: 8 cores = 2 batches x 4 head-groups (4 heads each); outputs are
disjoint -> no collectives.

Per-core structure: loop over five 512-token chunks J (last is 256), with the
attention for chunk J's q-tiles interleaved right after its projections
(causality: those q-tiles only need k/v tiles <= J), so TensorE never drains.
  A_qk(J): q^T/k^T projected directly in [dk, token] layout via fp8-e4m3
    DoubleRow matmuls (stationary = the head's weight chunk-pair, moving =
    x^T; fp8 on q/k adds ~0.1% error while V stays bf16 — V dominates the
    output error budget). L2 norms via an all-ones [128,128] stationary
    matmul on the squared values (column sums broadcast to all partitions),
    1/sqrt as Exp(-0.5*Ln) so every ACT function lives in one LUT table set
    (a monkeypatch pins table selection; Sqrt would force table thrashing).
  A_v(J): v in token-major layout, bf16 matmuls; per-token norm on the free
    axis via DVE square+reduce; stored with a ones column appended so
    attn @ [v|1] yields the softmax denominator inside the same PSUM
    accumulation group.
  B(J): scores computed transposed (s^T[kt, qt]) so the exp blocks feed
    attn@v as the stationary operand directly. q,k are unit vectors =>
    |s| <= scale => no max subtraction needed; scale_h applied for free via
    the Exp activation's scale argument. 4 consecutive kt-blocks pack one
    PSUM bank -> one [128,<=512] Exp per group (ACT per-op overhead is the
    phase-B co-bottleneck).

Measured on the 8-core trn2 chip: ~358 us HW exec, rel err ~5e-3 (gate 2e-2).
"""

import numpy as np
import ml_dtypes

B = 2
H = 16
T = 2304
D = 2048
DK = 128
NT = T // 128   # 18 token tiles; tiles 0 and 17 are the state segments
NCH = D // 128  # 16 contraction chunks
HG = 4          # heads per core
N_CORES = 8
NJ = 5          # 512-token chunks (last is 256)

_NC = None


def build_nc():
    global _NC
    if _NC is not None:
        return _NC

    from contextlib import ExitStack

    import concourse.tile as tile
    from concourse import bacc, mybir
    from concourse.masks import make_upper_triangular

    bf16 = mybir.dt.bfloat16
    f32 = mybir.dt.float32
    fp8 = mybir.dt.float8e4
    DR = mybir.MatmulPerfMode.DoubleRow
    AF = mybir.ActivationFunctionType

    import concourse.bacc as _bacc_mod
    import concourse.hw_specs as _hw_specs

    if not getattr(_hw_specs, "_act_tables_patched", False):
        _orig_gat = _hw_specs.get_activation_tables

        def _gat_one_set(arch):
            t = _orig_gat(arch)
            out, seen = {}, False
            for nm, funcs in t.items():
                if nm == "natural_log_exp_and_others":
                    seen = True
                    out[nm] = funcs
                else:
                    out[nm] = funcs if seen else set()
            return out

        _hw_specs.get_activation_tables = _gat_one_set
        _bacc_mod.get_activation_tables = _gat_one_set
        _hw_specs._act_tables_patched = True

    nc = bacc.Bacc()
    xt = nc.declare_dram_parameter("xt", [128, NCH // 2, 2, T], fp8, isOutput=False)
    w_params = {
        name: nc.declare_dram_parameter(name, [128, NCH // 2, 2, HG * 128], fp8,
                                        isOutput=False)
        for name in ("wq", "wk", "wqs", "wks")
    }
    w_params.update({
        name: nc.declare_dram_parameter(name, [128, NCH, HG * 128], bf16,
                                        isOutput=False)
        for name in ("wv", "wvs")
    })
    xtv = nc.declare_dram_parameter("xtv", [128, NT, NCH, 128], bf16,
                                    isOutput=False)
    scal = nc.declare_dram_parameter("scal", [128, HG], f32, isOutput=False)
    out = nc.declare_dram_parameter("out", [HG, T, DK], f32, isOutput=True)

    with tile.TileContext(nc) as tc, ExitStack() as ctx:
        consts = ctx.enter_context(tc.tile_pool(name="consts", bufs=1))
        # triT[i, j] = 1 iff i <= j: allowed entries of a transposed-scores
        # diagonal block (kt_local <= qt_local)
        triT = consts.tile([128, 128], bf16)
        make_upper_triangular(nc, triT[:], val=1.0, diag=True)
        ones128 = consts.tile([128, 128], bf16)
        nc.vector.memset(ones128[:], 1.0)
        scal_s = consts.tile([128, HG], f32)
        nc.sync.dma_start(scal_s[:], scal[:])

        xpool0 = ctx.enter_context(tc.tile_pool(name="x", bufs=3))
        xs0 = xpool0.tile([128, NCH // 2, 2, 512], fp8, tag="xs")
        nc.sync.dma_start(xs0[:], xt[:, :, :, 0:512])

        wpool = ctx.enter_context(tc.tile_pool(name="w", bufs=1))
        w_s = {}
        for name in ("wq", "wk", "wqs", "wks"):
            w_s[name] = wpool.tile([128, NCH // 2, 2, HG * 128], fp8, tag=name,
                                   name=name)
        for name in ("wv", "wvs"):
            w_s[name] = wpool.tile([128, NCH, HG * 128], bf16, tag=name,
                                   name=name)
        for pair in (("wqs", "wq"), ("wks", "wk")):
            for c2 in range(0, NCH // 2, 2):
                for name in pair:
                    nc.sync.dma_start(w_s[name][:, c2:c2 + 2],
                                      w_params[name][:, c2:c2 + 2])
        for name in ("wvs", "wv"):
            for c4 in range(0, NCH, 4):
                nc.sync.dma_start(w_s[name][:, c4:c4 + 4],
                                  w_params[name][:, c4:c4 + 4])

        big = ctx.enter_context(tc.tile_pool(name="big", bufs=1))
        kT_s = big.tile([128, HG, T], bf16, tag="kT")          # [dk, h, t]
        v_s = big.tile([128, HG, NT, 129], bf16, tag="v")      # [tl, h, ti, dv|1]
        nc.vector.memset(v_s[:, :, :, 128:129], 1.0)

        xpool = xpool0
        xvpool = ctx.enter_context(tc.tile_pool(name="xv", bufs=2))
        qpool = ctx.enter_context(tc.tile_pool(name="q", bufs=2))

        with (
            tc.tile_pool(name="psA", bufs=3, space="PSUM") as psA,
            tc.tile_pool(name="misc", bufs=3, space="PSUM") as misc,
            tc.tile_pool(name="psO", bufs=2, space="PSUM") as psO,
            tc.tile_pool(name="wk", bufs=3) as wk,
            tc.tile_pool(name="exw", bufs=6) as exw,
            tc.tile_pool(name="outp", bufs=3) as outp,
        ):
            for J in range(NJ):
                ntj = 4 if J < 4 else 2
                W = ntj * 128
                # x chunk: [128=d%128, tile, c, tl]
                if J == 0:
                    xs = xs0
                else:
                    xs = xpool.tile([128, NCH // 2, 2, 512], fp8, tag="xs")
                    nc.sync.dma_start(xs[:, :, :, :W],
                                      xt[:, :, :, 512 * J:512 * J + W])
                xv = xvpool.tile([128, 4, NCH, 128], bf16, tag="xv")
                nc.sync.dma_start(xv[:, :ntj], xtv[:, 4 * J:4 * J + ntj])
                qT_j = qpool.tile([128, HG, 512], bf16, tag="qT")

                state_w = {0: True, NJ - 1: True}.get(J, False)
                # tiles 0 and 17 are state; J=0 has tiles 0..3 (mixed), J=4 has
                # 16,17 (mixed) -> choose weights per token tile for v; for q/k
                # the 512-wide matmul spans tiles with different weights, so
                # split the contraction per token tile instead: do matmuls per
                # (c, tile-group of same kind).

                # ---- A_qk: transposed projections + matmul-based norms ----
                for kind, wc, ws, dest in (("q", "wq", "wqs", None),
                                           ("k", "wk", "wks", None)):
                    for h in range(HG):
                        ps = psA.tile([128, 512], f32, tag="proj")
                        for c in range(NCH // 2):
                            for lo, hi, wname in _wgroups(J, ntj, wc, ws):
                                nc.tensor.matmul(
                                    ps[:, lo * 128:hi * 128],
                                    w_s[wname][:, c, :, h * 128:(h + 1) * 128],
                                    xs[:, c, :, lo * 128:hi * 128],
                                    start=(c == 0), stop=(c == NCH // 2 - 1),
                                    perf_mode=DR,
                                )
                        pc = wk.tile([128, 512], bf16, tag="pc")
                        nc.vector.tensor_copy(pc[:, :W], ps[:, :W])
                        sq = wk.tile([128, 512], bf16, tag="sq")
                        nc.vector.tensor_mul(sq[:, :W], pc[:, :W], pc[:, :W])
                        n2 = misc.tile([128, 512], f32, tag="mb")
                        nc.tensor.matmul(n2[:, :W], ones128[:], sq[:, :W],
                                         start=True, stop=True)
                        sn = wk.tile([128, 512], f32, tag="sn")
                        nc.scalar.activation(sn[:, :W], n2[:, :W], AF.Ln)
                        nc.scalar.activation(sn[:, :W], sn[:, :W], AF.Exp,
                                             scale=-0.5)
                        tgt = (qT_j[:, h, :W] if kind == "q"
                               else kT_s[:, h, 512 * J:512 * J + W])
                        nc.vector.tensor_mul(tgt, pc[:, :W], sn[:, :W])

                # ---- A_v: token-major v + ones column ----
                for tl in range(ntj):
                    i = 4 * J + tl
                    wname = "wvs" if i in (0, NT - 1) else "wv"
                    pv = psA.tile([128, 512], f32, tag="proj")
                    for c in range(NCH):
                        nc.tensor.matmul(
                            pv[:], xv[:, tl, c, :], w_s[wname][:, c],
                            start=(c == 0), stop=(c == NCH - 1),
                        )
                    vc = wk.tile([128, 512], bf16, tag="vc")
                    nc.vector.tensor_copy(vc[:], pv[:])
                    sqv = wk.tile([128, 512], f32, tag="sqv")
                    nc.vector.tensor_mul(sqv[:], vc[:], vc[:])
                    n2v = wk.tile([128, HG], f32, tag="n2v")
                    nc.vector.reduce_sum(
                        out=n2v[:],
                        in_=sqv[:].rearrange("p (h e) -> p h e", h=HG),
                        axis=mybir.AxisListType.X,
                    )
                    nc.scalar.activation(n2v[:], n2v[:], AF.Ln)
                    nc.scalar.activation(n2v[:], n2v[:], AF.Exp, scale=-0.5)
                    for h in range(HG):
                        nc.vector.tensor_scalar_mul(
                            v_s[:, h, i, 0:128],
                            vc[:, h * 128:(h + 1) * 128],
                            n2v[:, h:h + 1],
                        )

                # ---- B: attention for q-tiles of this chunk ----
                for ql in range(ntj):
                    for h in range(HG):
                        qi = 4 * J + ql
                        po = psO.tile([128, 129], f32, tag="po")
                        kis = [ki for ki in range(qi + 1)
                               if not (qi == NT - 1 and ki == 0)]
                        ngroups = qi // 4 + 1
                        first_av = True
                        for g in range(ngroups):
                            gkis = [ki for ki in range(4 * g, min(4 * g + 4, qi + 1))]
                            gw = len(gkis) * 128
                            pss = misc.tile([128, 512], f32, tag="mb")
                            for sl, ki in enumerate(gkis):
                                nc.tensor.matmul(
                                    pss[:, sl * 128:(sl + 1) * 128],
                                    kT_s[:, h, ki * 128:(ki + 1) * 128],
                                    qT_j[:, h, ql * 128:(ql + 1) * 128],
                                    start=True, stop=True,
                                )
                            ex = exw.tile([128, 512], bf16, tag="ex")
                            nc.scalar.activation(ex[:, :gw], pss[:, :gw], AF.Exp,
                                                 scale=scal_s[:, h:h + 1])
                            if gkis[-1] == qi:
                                sl = len(gkis) - 1
                                nc.vector.tensor_mul(
                                    ex[:, sl * 128:(sl + 1) * 128],
                                    ex[:, sl * 128:(sl + 1) * 128],
                                    triT[:],
                                )
                            for sl, ki in enumerate(gkis):
                                if ki not in kis:
                                    continue
                                nc.tensor.matmul(
                                    po[:], ex[:, sl * 128:(sl + 1) * 128],
                                    v_s[:, h, ki, :],
                                    start=first_av, stop=(ki == kis[-1]),
                                )
                                first_av = False
                        den = exw.tile([128, 1], f32, tag="den")
                        nc.vector.reciprocal(den[:], po[:, 128:129])
                        ot = outp.tile([128, 128], f32, tag="ot")
                        nc.vector.tensor_scalar_mul(ot[:], po[:, 0:128], den[:])
                        nc.sync.dma_start(out[h, qi * 128:(qi + 1) * 128, :], ot[:])

    nc.finalize()
    _NC = nc
    return nc


def _wgroups(J, ntj, wc, ws):
    """Token-tile ranges [lo, hi) within chunk J sharing one weight tensor."""
    if J == 0:
        return [(0, 1, ws), (1, ntj, wc)]
    if J == NJ - 1:
        return [(0, ntj - 1, wc), (ntj - 1, ntj, ws)]
    return [(0, ntj, wc)]


def _shard_inputs(x, w_q, w_k, w_v, w_q_state, w_k_state, w_v_state,
                  scaling_factor):
    bf16 = ml_dtypes.bfloat16

    fp8 = ml_dtypes.float8_e4m3

    def prep_x(xb):
        xt = np.ascontiguousarray(xb.T)                       # (D, T)
        xt = xt.reshape(NCH // 2, 2, 128, T).transpose(2, 0, 1, 3)
        return np.ascontiguousarray(xt.astype(fp8))

    def prep_xv(xb):
        xt = np.ascontiguousarray(xb.T)
        xt = xt.reshape(NCH, 128, NT, 128).transpose(1, 2, 0, 3)
        return np.ascontiguousarray(xt.astype(bf16))

    def prep_w(w, g):
        w4 = w[HG * g:HG * (g + 1)].transpose(1, 0, 2).reshape(D, HG * DK)
        w4 = w4.reshape(NCH // 2, 2, 128, HG * DK).transpose(2, 0, 1, 3)
        return np.ascontiguousarray(w4.astype(fp8))

    def prep_wv(w, g):
        w4 = w[HG * g:HG * (g + 1)].transpose(1, 0, 2).reshape(D, HG * DK)
        w4 = w4.reshape(NCH, 128, HG * DK).transpose(1, 0, 2)
        return np.ascontiguousarray(w4.astype(bf16))

    xbs = [np.asarray(x[b], dtype=np.float32) for b in range(B)]
    xts = [prep_x(xb) for xb in xbs]
    xtvs = [prep_xv(xb) for xb in xbs]
    in_maps = []
    for core in range(N_CORES):
        b, g = divmod(core, N_CORES // B)
        m = {"xt": xts[b], "xtv": xtvs[b]}
        for name, w in (("wq", w_q), ("wk", w_k),
                        ("wqs", w_q_state), ("wks", w_k_state)):
            m[name] = prep_w(np.asarray(w, dtype=np.float32), g)
        for name, w in (("wv", w_v), ("wvs", w_v_state)):
            m[name] = prep_wv(np.asarray(w, dtype=np.float32), g)
        sc = np.asarray(scaling_factor, dtype=np.float32)[HG * g:HG * (g + 1)]
        m["scal"] = np.ascontiguousarray(
            np.broadcast_to(sc[None, :], (128, HG)).astype(np.float32))
        in_maps.append(m)
    return in_maps


def run_on_cores(in_maps, **kwargs):
    from concourse.bass_utils import run_bass_kernel_spmd

    nc = build_nc()
    return run_bass_kernel_spmd(nc, in_maps, list(range(N_CORES)), **kwargs)


def kernel(x, w_q, w_k, w_v, w_q_state, w_k_state, w_v_state, scaling_factor):
    in_maps = _shard_inputs(x, w_q, w_k, w_v, w_q_state, w_k_state, w_v_state,
                            scaling_factor)
    res = run_on_cores(in_maps)
    full = np.empty((B, H, T, DK), dtype=np.float32)
    for core in range(N_CORES):
        b, g = divmod(core, N_CORES // B)
        full[b, HG * g:HG * (g + 1)] = res.results[core]["out"]
    return full


# revision 24
# speedup vs baseline: 1.5086x; 1.2571x over previous
"""Trainium2 Bass kernel for stacked-head state attention (nn_ARCStacked).

Problem (hardcoded shapes):
  x: (2, 2304, 2048) f32; six weights (16, 2048, 128) f32; scaling (16,) f32.
  T = 2304 = 128 state + 2048 core + 128 state tokens. Per (batch b, head h):
    q/k/v = l2norm(x @ w[h]) per token (state weights for the two 128-token
    state segments, core weights for the middle 2048), causal attention with
    the extra rule that the last 128 rows can't attend to the first 128 cols,
    out = softmax(scale_h * q @ k^T + mask) @ v  -> (2, 16, 2304, 128) f32.

Sharding: 8 cores = 2 batches x 4 head-groups (4 heads each); outputs are
disjoint -> no collectives.

Per-core structure: loop over five 512-token chunks J (last is 256), with the
attention for chunk J's q-tiles interleaved right after its projections
(causality: those q-tiles only need k/v tiles <= J), so TensorE never drains.
  A_qk(J): q^T/k^T projected directly in [dk, token] layout via fp8-e4m3
    DoubleRow matmuls (stationary = the head's weight chunk-pair, moving =
    x^T; fp8 on q/k adds ~0.1% error while V stays bf16 — V dominates the
    output error budget). L2 norms via an all-ones [128,128] stationary
    matmul on the squared values (column sums broadcast to all partitions),
    1/sqrt as Exp(-0.5*Ln) so every ACT function lives in one LUT table set
    (a monkeypatch pins table selection; Sqrt would force table thrashing).
  A_v(J): v in token-major layout, bf16 matmuls; per-token norm on the free
    axis via DVE square+reduce; stored with a ones column appended so
    attn @ [v|1] yields the softmax denominator inside the same PSUM
    accumulation group.
  B(J): scores computed transposed (s^T[kt, qt]) so the exp blocks feed
    attn@v as the stationary operand directly. q,k are unit vectors =>
    |s| <= scale => no max subtraction needed; scale_h applied for free via
    the Exp activation's scale argument. 4 consecutive kt-blocks pack one
    PSUM bank -> one [128,<=512] Exp per group (ACT per-op overhead is the
    phase-B co-bottleneck).

Measured on the 8-core trn2 chip: ~358 us HW exec, rel err ~5e-3 (gate 2e-2).
"""

import numpy as np
import ml_dtypes

B = 2
H = 16
T = 2304
D = 2048
DK = 128
NT = T // 128   # 18 token tiles; tiles 0 and 17 are the state segments
NCH = D // 128  # 16 contraction chunks
HG = 4          # heads per core
N_CORES = 8
NJ = 5          # 512-token chunks (last is 256)

_NC = None


def build_nc():
    global _NC
    if _NC is not None:
        return _NC

    from contextlib import ExitStack

    import concourse.tile as tile
    from concourse import bacc, mybir
    from concourse.masks import make_upper_triangular

    bf16 = mybir.dt.bfloat16
    f32 = mybir.dt.float32
    fp8 = mybir.dt.float8e4
    DR = mybir.MatmulPerfMode.DoubleRow
    AF = mybir.ActivationFunctionType

    import concourse.bacc as _bacc_mod
    import concourse.hw_specs as _hw_specs

    if not getattr(_hw_specs, "_act_tables_patched", False):
        _orig_gat = _hw_specs.get_activation_tables

        def _gat_one_set(arch):
            t = _orig_gat(arch)
            out, seen = {}, False
            for nm, funcs in t.items():
                if nm == "natural_log_exp_and_others":
                    seen = True
                    out[nm] = funcs
                else:
                    out[nm] = funcs if seen else set()
            return out

        _hw_specs.get_activation_tables = _gat_one_set
        _bacc_mod.get_activation_tables = _gat_one_set
        _hw_specs._act_tables_patched = True

    nc = bacc.Bacc()
    xt = nc.declare_dram_parameter("xt", [128, NCH // 2, 2, T], fp8, isOutput=False)
    w_params = {
        name: nc.declare_dram_parameter(name, [128, NCH // 2, 2, HG * 128], fp8,
                                        isOutput=False)
        for name in ("wq", "wk", "wqs", "wks")
    }
    w_params.update({
        name: nc.declare_dram_parameter(name, [128, NCH, HG * 128], bf16,
                                        isOutput=False)
        for name in ("wv", "wvs")
    })
    xtv = nc.declare_dram_parameter("xtv", [128, NT, NCH, 128], bf16,
                                    isOutput=False)
    scal = nc.declare_dram_parameter("scal", [128, HG], f32, isOutput=False)
    out = nc.declare_dram_parameter("out", [HG, T, DK], f32, isOutput=True)

    with tile.TileContext(nc) as tc, ExitStack() as ctx:
        consts = ctx.enter_context(tc.tile_pool(name="consts", bufs=1))
        # triT[i, j] = 1 iff i <= j: allowed entries of a transposed-scores
        # diagonal block (kt_local <= qt_local)
        triT = consts.tile([128, 128], bf16)
        make_upper_triangular(nc, triT[:], val=1.0, diag=True)
        ones128 = consts.tile([128, 128], bf16)
        nc.vector.memset(ones128[:], 1.0)
        scal_s = consts.tile([128, HG], f32)
        nc.sync.dma_start(scal_s[:], scal[:])

        xpool0 = ctx.enter_context(tc.tile_pool(name="x", bufs=3))
        xvpool = ctx.enter_context(tc.tile_pool(name="xv", bufs=2))
        xs0 = xpool0.tile([128, NCH // 2, 2, 512], fp8, tag="xs")
        nc.sync.dma_start(xs0[:], xt[:, :, :, 0:512])
        xv0 = xvpool.tile([128, 4, NCH, 128], bf16, tag="xv")
        nc.sync.dma_start(xv0[:], xtv[:, 0:4])

        wpool = ctx.enter_context(tc.tile_pool(name="w", bufs=1))
        w_s = {}
        for name in ("wq", "wk", "wqs", "wks"):
            w_s[name] = wpool.tile([128, NCH // 2, 2, HG * 128], fp8, tag=name,
                                   name=name)
        for name in ("wv", "wvs"):
            w_s[name] = wpool.tile([128, NCH, HG * 128], bf16, tag=name,
                                   name=name)
        for pair in (("wqs", "wq"), ("wks", "wk")):
            for c2 in range(0, NCH // 2, 2):
                for name in pair:
                    nc.sync.dma_start(w_s[name][:, c2:c2 + 2],
                                      w_params[name][:, c2:c2 + 2])
        for name in ("wvs", "wv"):
            for c4 in range(0, NCH, 4):
                nc.sync.dma_start(w_s[name][:, c4:c4 + 4],
                                  w_params[name][:, c4:c4 + 4])

        big = ctx.enter_context(tc.tile_pool(name="big", bufs=1))
        kT_s = big.tile([128, HG, T], bf16, tag="kT")          # [dk, h, t]
        v_s = big.tile([128, HG, NT, 129], bf16, tag="v")      # [tl, h, ti, dv|1]
        nc.vector.memset(v_s[:, :, :, 128:129], 1.0)

        xpool = xpool0
        qpool = ctx.enter_context(tc.tile_pool(name="q", bufs=2))

        with (
            tc.tile_pool(name="psA", bufs=3, space="PSUM") as psA,
            tc.tile_pool(name="misc", bufs=3, space="PSUM") as misc,
            tc.tile_pool(name="psO", bufs=2, space="PSUM") as psO,
            tc.tile_pool(name="wk", bufs=3) as wk,
            tc.tile_pool(name="exw", bufs=8) as exw,
            tc.tile_pool(name="outp", bufs=4) as outp,
        ):
            cur = (xs0, xv0)
            for J in range(NJ):
                ntj = 4 if J < 4 else 2
                W = ntj * 128
                xs, xv = cur
                if J + 1 < NJ:
                    ntn = 4 if J + 1 < 4 else 2
                    xsn = xpool.tile([128, NCH // 2, 2, 512], fp8, tag="xs",
                                     name=f"xs{J + 1}")
                    nc.sync.dma_start(
                        xsn[:, :, :, :ntn * 128],
                        xt[:, :, :, 512 * (J + 1):512 * (J + 1) + ntn * 128])
                    xvn = xvpool.tile([128, 4, NCH, 128], bf16, tag="xv",
                                      name=f"xv{J + 1}")
                    nc.sync.dma_start(xvn[:, :ntn],
                                      xtv[:, 4 * (J + 1):4 * (J + 1) + ntn])
                    cur = (xsn, xvn)
                qT_j = qpool.tile([128, HG, 512], bf16, tag="qT")

                state_w = {0: True, NJ - 1: True}.get(J, False)
                # tiles 0 and 17 are state; J=0 has tiles 0..3 (mixed), J=4 has
                # 16,17 (mixed) -> choose weights per token tile for v; for q/k
                # the 512-wide matmul spans tiles with different weights, so
                # split the contraction per token tile instead: do matmuls per
                # (c, tile-group of same kind).

                # ---- A_qk: transposed projections + matmul-based norms ----
                for kind, wc, ws, dest in (("q", "wq", "wqs", None),
                                           ("k", "wk", "wks", None)):
                    for h in range(HG):
                        ps = psA.tile([128, 512], f32, tag="proj")
                        for c in range(NCH // 2):
                            for lo, hi, wname in _wgroups(J, ntj, wc, ws):
                                nc.tensor.matmul(
                                    ps[:, lo * 128:hi * 128],
                                    w_s[wname][:, c, :, h * 128:(h + 1) * 128],
                                    xs[:, c, :, lo * 128:hi * 128],
                                    start=(c == 0), stop=(c == NCH // 2 - 1),
                                    perf_mode=DR,
                                )
                        pc = wk.tile([128, 512], bf16, tag="pc")
                        nc.vector.tensor_copy(pc[:, :W], ps[:, :W])
                        sq = wk.tile([128, 512], bf16, tag="sq")
                        nc.vector.tensor_mul(sq[:, :W], pc[:, :W], pc[:, :W])
                        n2 = misc.tile([128, 512], f32, tag="mb")
                        nc.tensor.matmul(n2[:, :W], ones128[:], sq[:, :W],
                                         start=True, stop=True)
                        sn = wk.tile([128, 512], f32, tag="sn")
                        nc.scalar.activation(sn[:, :W], n2[:, :W], AF.Ln)
                        nc.scalar.activation(sn[:, :W], sn[:, :W], AF.Exp,
                                             scale=-0.5)
                        tgt = (qT_j[:, h, :W] if kind == "q"
                               else kT_s[:, h, 512 * J:512 * J + W])
                        nc.vector.tensor_mul(tgt, pc[:, :W], sn[:, :W])

                # ---- A_v: token-major v + ones column ----
                for tl in range(ntj):
                    i = 4 * J + tl
                    wname = "wvs" if i in (0, NT - 1) else "wv"
                    pv = psA.tile([128, 512], f32, tag="proj")
                    for c in range(NCH):
                        nc.tensor.matmul(
                            pv[:], xv[:, tl, c, :], w_s[wname][:, c],
                            start=(c == 0), stop=(c == NCH - 1),
                        )
                    vc = wk.tile([128, 512], bf16, tag="vc")
                    nc.vector.tensor_copy(vc[:], pv[:])
                    sqv = wk.tile([128, 512], f32, tag="sqv")
                    nc.vector.tensor_mul(sqv[:], vc[:], vc[:])
                    n2v = wk.tile([128, HG], f32, tag="n2v")
                    nc.vector.reduce_sum(
                        out=n2v[:],
                        in_=sqv[:].rearrange("p (h e) -> p h e", h=HG),
                        axis=mybir.AxisListType.X,
                    )
                    nc.scalar.activation(n2v[:], n2v[:], AF.Ln)
                    nc.scalar.activation(n2v[:], n2v[:], AF.Exp, scale=-0.5)
                    for h in range(HG):
                        nc.vector.tensor_scalar_mul(
                            v_s[:, h, i, 0:128],
                            vc[:, h * 128:(h + 1) * 128],
                            n2v[:, h:h + 1],
                        )

                # ---- B: attention for q-tiles of this chunk ----
                for ql in range(ntj):
                    for h in range(HG):
                        qi = 4 * J + ql
                        po = psO.tile([128, 129], f32, tag="po")
                        kis = [ki for ki in range(qi + 1)
                               if not (qi == NT - 1 and ki == 0)]
                        ngroups = qi // 4 + 1
                        first_av = True
                        for g in range(ngroups):
                            gkis = [ki for ki in range(4 * g, min(4 * g + 4, qi + 1))]
                            gw = len(gkis) * 128
                            pss = misc.tile([128, 512], f32, tag="mb")
                            for sl, ki in enumerate(gkis):
                                nc.tensor.matmul(
                                    pss[:, sl * 128:(sl + 1) * 128],
                                    kT_s[:, h, ki * 128:(ki + 1) * 128],
                                    qT_j[:, h, ql * 128:(ql + 1) * 128],
                                    start=True, stop=True,
                                )
                            ex = exw.tile([128, 512], bf16, tag="ex")
                            nc.scalar.activation(ex[:, :gw], pss[:, :gw], AF.Exp,
                                                 scale=scal_s[:, h:h + 1])
                            if gkis[-1] == qi:
                                sl = len(gkis) - 1
                                nc.vector.tensor_mul(
                                    ex[:, sl * 128:(sl + 1) * 128],
                                    ex[:, sl * 128:(sl + 1) * 128],
                                    triT[:],
                                )
                            for sl, ki in enumerate(gkis):
                                if ki not in kis:
                                    continue
                                nc.tensor.matmul(
                                    po[:], ex[:, sl * 128:(sl + 1) * 128],
                                    v_s[:, h, ki, :],
                                    start=first_av, stop=(ki == kis[-1]),
                                )
                                first_av = False
                        den = exw.tile([128, 1], f32, tag="den")
                        nc.vector.reciprocal(den[:], po[:, 128:129])
                        ot = outp.tile([128, 128], f32, tag="ot")
                        nc.vector.tensor_scalar_mul(ot[:], po[:, 0:128], den[:])
                        nc.sync.dma_start(out[h, qi * 128:(qi + 1) * 128, :], ot[:])

    nc.finalize()
    _NC = nc
    return nc


def _wgroups(J, ntj, wc, ws):
    """Token-tile ranges [lo, hi) within chunk J sharing one weight tensor."""
    if J == 0:
        return [(0, 1, ws), (1, ntj, wc)]
    if J == NJ - 1:
        return [(0, ntj - 1, wc), (ntj - 1, ntj, ws)]
    return [(0, ntj, wc)]


def _shard_inputs(x, w_q, w_k, w_v, w_q_state, w_k_state, w_v_state,
                  scaling_factor):
    bf16 = ml_dtypes.bfloat16

    fp8 = ml_dtypes.float8_e4m3

    def prep_x(xb):
        xt = np.ascontiguousarray(xb.T)                       # (D, T)
        xt = xt.reshape(NCH // 2, 2, 128, T).transpose(2, 0, 1, 3)
        return np.ascontiguousarray(xt.astype(fp8))

    def prep_xv(xb):
        xt = np.ascontiguousarray(xb.T)
        xt = xt.reshape(NCH, 128, NT, 128).transpose(1, 2, 0, 3)
        return np.ascontiguousarray(xt.astype(bf16))

    def prep_w(w, g):
        w4 = w[HG * g:HG * (g + 1)].transpose(1, 0, 2).reshape(D, HG * DK)
        w4 = w4.reshape(NCH // 2, 2, 128, HG * DK).transpose(2, 0, 1, 3)
        return np.ascontiguousarray(w4.astype(fp8))

    def prep_wv(w, g):
        w4 = w[HG * g:HG * (g + 1)].transpose(1, 0, 2).reshape(D, HG * DK)
        w4 = w4.reshape(NCH, 128, HG * DK).transpose(1, 0, 2)
        return np.ascontiguousarray(w4.astype(bf16))

    xbs = [np.asarray(x[b], dtype=np.float32) for b in range(B)]
    xts = [prep_x(xb) for xb in xbs]
    xtvs = [prep_xv(xb) for xb in xbs]
    in_maps = []
    for core in range(N_CORES):
        b, g = divmod(core, N_CORES // B)
        m = {"xt": xts[b], "xtv": xtvs[b]}
        for name, w in (("wq", w_q), ("wk", w_k),
                        ("wqs", w_q_state), ("wks", w_k_state)):
            m[name] = prep_w(np.asarray(w, dtype=np.float32), g)
        for name, w in (("wv", w_v), ("wvs", w_v_state)):
            m[name] = prep_wv(np.asarray(w, dtype=np.float32), g)
        sc = np.asarray(scaling_factor, dtype=np.float32)[HG * g:HG * (g + 1)]
        m["scal"] = np.ascontiguousarray(
            np.broadcast_to(sc[None, :], (128, HG)).astype(np.float32))
        in_maps.append(m)
    return in_maps


def run_on_cores(in_maps, **kwargs):
    from concourse.bass_utils import run_bass_kernel_spmd

    nc = build_nc()
    return run_bass_kernel_spmd(nc, in_maps, list(range(N_CORES)), **kwargs)


def _run_and_gather(in_maps):
    res = run_on_cores(in_maps)
    full = np.empty((B, H, T, DK), dtype=np.float32)
    for core in range(N_CORES):
        b, g = divmod(core, N_CORES // B)
        full[b, HG * g:HG * (g + 1)] = res.results[core]["out"]
    return full


def kernel(x, w_q, w_k, w_v, w_q_state, w_k_state, w_v_state, scaling_factor):
    import os

    in_maps = _shard_inputs(x, w_q, w_k, w_v, w_q_state, w_k_state, w_v_state,
                            scaling_factor)
    try:
        return _run_and_gather(in_maps)
    except Exception:
        if os.environ.get("KERNEL_NO_RETRY") == "1":
            raise
    # The device occasionally hits a transient NRT_EXEC_UNIT_UNRECOVERABLE;
    # a fresh process (fresh PJRT client) recovers it, and the neuron compile
    # cache makes the retry cheap. Ship the raw inputs to the subprocess.
    import subprocess
    import sys
    import tempfile

    names = ("x", "w_q", "w_k", "w_v", "w_q_state", "w_k_state", "w_v_state",
             "scaling_factor")
    vals = (x, w_q, w_k, w_v, w_q_state, w_k_state, w_v_state, scaling_factor)
    here = os.path.dirname(os.path.abspath(__file__))
    last_err = None
    for _attempt in range(3):
        with tempfile.TemporaryDirectory() as td:
            inp = os.path.join(td, "in.npz")
            outp = os.path.join(td, "out.npy")
            np.savez(inp, **{n: np.asarray(v, dtype=np.float32)
                             for n, v in zip(names, vals)})
            code = (
                "import sys, numpy as np\n"
                f"sys.path.insert(0, {here!r})\n"
                "import kernel\n"
                f"d = np.load({inp!r})\n"
                "out = kernel.kernel(**{k: d[k] for k in d.files})\n"
                f"np.save({outp!r}, out)\n"
            )
            env = dict(os.environ)
            env["KERNEL_NO_RETRY"] = "1"
            r = subprocess.run([sys.executable, "-c", code], env=env)
            if r.returncode == 0 and os.path.exists(outp):
                return np.load(outp)
            last_err = RuntimeError(
                f"kernel subprocess retry failed rc={r.returncode}")
    raise last_err
